# revision 34
# baseline (speedup 1.0000x reference)
"""Trainium2 Bass kernel for nn_HardcodedKVMemoryBlock (8 NeuronCores).

Sharding: core i handles batch b=i//4, sequence chunk c=i%4 (512 tokens).
The (B,L,P,D) cumsum is restructured as causal linear attention:
    retrieved = tril(Q K^T) @ V + Q @ S_prefix
with Q=[cos,sin] phasors (L x 64), V = values at odd positions, and the
cross-chunk carry S_prefix computed redundantly per core from a
zero-padded prefix (no collectives). The 1/sqrt(valid*P) normalization
cancels inside LayerNorm1 (scale invariance); ln gains are folded into
W1/Wo and means are applied as rank-1 PE updates.

v3 (structural rework of the v1 baseline):
 - x and the prefix-even tokens are pre-transposed on the host, removing
   all 20 PE layout transposes + psum copies of v1.
 - squares for LN stats run on DVE/GpSimd (Square table loads gone).
 - activation tables (tanh/gelu) are prefetched with dummy ACTs so no
   table load sits on the critical path.
 - cos = sin(-pi*|t| + pi/2) via per-partition scale/bias APs, removing
   the affine DVE pass of v1.
 - rsqrt Newton iterations run on GpSimd (DVE freed).
 - dummy warmup matmuls keep the PE busy during the input DMA so the
   tensor engine p-state is ramped when real work arrives.
"""

import math
import numpy as np
import ml_dtypes

import concourse.bass as bass
import concourse.tile as tile
from concourse import bacc, mybir
from concourse.bass_utils import run_bass_kernel_spmd

PI = math.pi
B, L, D, P = 2, 2048, 256, 32
T = 512          # own tokens per core
H = 512          # MLP hidden
PRE = 768        # padded prefix pair count (max prefix 1536 tokens / 2)
N_CORES = 8

f32 = mybir.dt.float32
f32r = mybir.dt.float32r
bf16 = mybir.dt.bfloat16
AF = mybir.ActivationFunctionType
OP = mybir.AluOpType


def _r(ap):
    return ap.bitcast(f32r)


N_WARM = 3       # dummy PE warmup matmuls


def _build():
    nc = bacc.Bacc("TRN2", target_bir_lowering=False, debug=False,
                   num_devices=N_CORES)

    def din(name, shape, dt):
        return nc.dram_tensor(name, shape, dt, kind="ExternalInput").ap()

    # all inputs are stored partition-major on the host so every DMA is
    # 128 contiguous rows (minimal descriptors -> cheap issue, fast xfer)
    xc = din("xc", [128, 4, 256], f32)     # own tokens token-major (residual)
    xct = din("xct", [128, 2, 512], f32r)  # own tokens, transposed
    xpet = din("xpet", [128, 2, PRE], bf16)
    xpo = din("xpo", [128, 6, 256], bf16)
    kw = din("kw", [128, 2, P], f32r)
    kwb = din("kwb", [128, 2, P], bf16)
    vw = din("vw", [128, 2, 256], f32r)
    w1g = din("w1g", [128, 2, H], bf16)    # ln1_g . W1
    c1n = din("c1n", [1, H], bf16)         # -w1g.sum(0)
    cb1 = din("cb1", [128, 4], f32)        # b1 + ln1_b@W1, col layout
    w2 = din("w2", [128, 4, 256], bf16)
    b2c = din("b2c", [128, 2], f32)
    wog = din("wog", [128, 2, 256], bf16)  # ln2_g . Wo
    c1on = din("c1on", [1, D], bf16)       # -wog.sum(0)
    m0 = din("m0", [128, 256], bf16)       # causal mask for odd writes
    onesr = din("onesr", [1, 128], f32r)
    invdb = din("invdb", [128, 1], bf16)   # 1/D
    eyef = din("eyef", [128, 128], f32)
    eyeb = din("eyeb", [64, 64], bf16)
    outc = nc.dram_tensor("outc", [T, D], f32, kind="ExternalOutput").ap()

    with tile.TileContext(nc) as tc:
        _emit(tc, locals())
    nc.compile()
    return nc


def _emit(tc, io):
    nc = tc.nc
    outc = io["outc"]

    sb = tc.alloc_tile_pool(name="sb", bufs=1)
    pt = tc.alloc_tile_pool(name="pt", bufs=2, space="PSUM")    # small tiles
    pa = tc.alloc_tile_pool(name="pa", bufs=2, space="PSUM")    # narrow tiles
    pb = tc.alloc_tile_pool(name="pb", bufs=3, space="PSUM")    # [128,512]

    # ---------------- constant / weight tiles ----------------
    xc_sb = sb.tile([128, 4, 256], f32)
    xct_sb = sb.tile([128, 2, 512], f32r)
    xpet_sb = sb.tile([128, 2, PRE], bf16)
    xpo_sb = sb.tile([128, 6, 256], bf16)
    kw_sb = sb.tile([128, 2, P], f32r)
    kwb_sb = sb.tile([128, 2, P], bf16)
    vw_sb = sb.tile([128, 2, 256], f32r)
    w1g_sb = sb.tile([128, 2, H], bf16)
    c1n_sb = sb.tile([1, H], bf16)
    cb1_sb = sb.tile([128, 4], f32)
    w2_sb = sb.tile([128, 4, 256], bf16)
    b2c_sb = sb.tile([128, 2], f32)
    wog_sb = sb.tile([128, 2, 256], bf16)
    c1on_sb = sb.tile([1, D], bf16)
    m0_sb = sb.tile([128, 256], bf16)
    eyef_sb = sb.tile([128, 128], f32)
    eyeb_sb = sb.tile([64, 64], bf16)
    ones_sb = sb.tile([1, 128], f32r)
    invdb_sb = sb.tile([128, 1], bf16)
    qbias_sb = sb.tile([2 * P, 1], f32)    # pi/2 rows 0:32, 0 rows 32:64
    qscale_sb = sb.tile([2 * P, 1], f32)   # -pi rows 0:32, pi rows 32:64
    zero128_sb = sb.tile([128, 1], f32)
    halfpi_sb = sb.tile([128, 1], f32)
    dumm_sb = sb.tile([1, 1], f32)
    warm_sb = sb.tile([128, 512], bf16)

    dma = nc.sync.dma_start
    dmaw = nc.gpsimd.dma_start
    nc.vector.memset(qbias_sb[0:P, :], PI / 2)
    nc.vector.memset(qbias_sb[P:2 * P, :], 0.0)
    nc.vector.memset(qscale_sb[0:P, :], -PI)
    nc.vector.memset(qscale_sb[P:2 * P, :], PI)
    nc.vector.memset(zero128_sb[:], 0.0)
    nc.vector.memset(halfpi_sb[:], PI / 2)
    nc.vector.memset(dumm_sb[:], 0.0)
    nc.vector.memset(warm_sb[:], 0.0)

    mm = nc.tensor.matmul
    act = nc.scalar.activation
    tt_ = nc.vector.tensor_tensor
    tcp = nc.vector.tensor_copy
    ts_ = nc.vector.tensor_scalar
    stt = nc.vector.scalar_tensor_tensor
    gts = nc.gpsimd.tensor_scalar
    gstt = nc.gpsimd.scalar_tensor_tensor
    gtt = nc.gpsimd.tensor_tensor

    # preload the silu_and_others table set (contains silu+tanh+sin+abs+copy)
    # while DMAs run: a Silu dummy forces that exact set, so the whole
    # phasor pipeline below needs zero further table loads.
    act(dumm_sb[:], dumm_sb[:], AF.Silu, bias=dumm_sb[:])

    # ---------------- data tiles ----------------
    tv_sb = sb.tile([2 * P, T], f32)       # rows 0:32 |tanh|, 32:64 tanh
    qb_sb = sb.tile([2 * P, T], f32r)       # rows 0:32 cos, 32:64 sin
    tvp_sb = sb.tile([2 * P, PRE], f32)
    kpre_sb = sb.tile([2 * P, PRE], bf16)
    kpreT_sb = sb.tile([128, 6, 64], bf16)
    gT_sb = sb.tile([128, 2, 64], f32r)
    s_sb = sb.tile([2 * P, D], f32r)        # prefix state S
    vodd_sb = sb.tile([128, 2, 256], f32r)  # values at odd tokens
    ss0_sb = sb.tile([128, 512], f32r)
    ss1_sb = sb.tile([128, 256], f32r)
    cross_sb = sb.tile([128, 2, 512], f32)
    r_sb = sb.tile([128, 2, 512], bf16)    # retrieved
    sq1_sb = sb.tile([128, 2, 512], bf16)
    mean1_sb = sb.tile([1, T], bf16)
    var1_sb = sb.tile([1, T], f32)
    m2_sb = sb.tile([1, T], f32)
    rstd1_sb = sb.tile([1, T], f32r)
    rb1s_sb = sb.tile([128, T], f32)
    hi_sb = sb.tile([128, 4, 512], bf16)
    h_sb = sb.tile([128, 4, 512], bf16)
    f_sb = sb.tile([128, 2, 512], bf16)    # refined
    sq2_sb = sb.tile([128, 2, 512], bf16)
    mean2_sb = sb.tile([1, T], bf16)
    var2_sb = sb.tile([1, T], f32)
    m2b_sb = sb.tile([1, T], f32)
    stdc_sb = sb.tile([128, 4], f32)
    rstdc_sb = sb.tile([128, 4], f32)
    stdc2_sb = sb.tile([128, 4], f32)
    rstdc2_sb = sb.tile([128, 4], f32)
    nwt_sb = sb.tile([128, 4], f32)
    out_sb = sb.tile([128, 4, 256], f32)

    from concourse.tile import add_dep_helper  # noqa: E402
    # critical-path loads first; prefix tensors issue from the gpsimd queue
    # xct split across 2 DMA rings - a single ring moves only ~90GB/s
    dma(xct_sb[:, 0, 0:256], io["xct"][:, 0, 0:256])
    dma(xct_sb[:, 0, 256:512], io["xct"][:, 0, 256:512])
    nc.scalar.dma_start(xct_sb[:, 1, 0:256], io["xct"][:, 1, 0:256])
    nc.scalar.dma_start(xct_sb[:, 1, 256:512], io["xct"][:, 1, 256:512])
    dma(kw_sb[:], io["kw"])
    dma(kwb_sb[:], io["kwb"])
    d_xpet = dmaw(xpet_sb[:], io["xpet"])
    dma(vw_sb[:], io["vw"])
    d_xpo = dmaw(xpo_sb[:], io["xpo"])
    dma(m0_sb[:], io["m0"])
    dma(invdb_sb[:], io["invdb"])
    dma(eyeb_sb[:], io["eyeb"])
    wd = []
    wd.append(dmaw(w1g_sb[:], io["w1g"]))
    wd.append(dmaw(c1n_sb[:], io["c1n"]))
    wd.append(dmaw(cb1_sb[:], io["cb1"]))
    wd.append(dmaw(w2_sb[:], io["w2"]))
    wd.append(dmaw(b2c_sb[:], io["b2c"]))
    wd.append(dmaw(wog_sb[:], io["wog"]))
    wd.append(dmaw(c1on_sb[:], io["c1on"]))
    wd.append(dmaw(eyef_sb[:], io["eyef"]))
    wd.append(dmaw(ones_sb[:], io["onesr"]))
    d_xc = dmaw(xc_sb[:], io["xc"])
    # bulk weights wait for the latency-critical input loads to finish so
    # they don't steal HBM bandwidth from the critical path
    for w in wd:
        add_dep_helper(w.ins, d_xpo.ins, sync=True,
                       reason="bulk weights after data loads")
    add_dep_helper(d_xc.ins, d_xpo.ins, sync=True,
                   reason="residual x after data loads")

    # ---------------- PE warmup (ramps the p-state during DMA) ----------
    for w in range(N_WARM):
        pw = pb.tile([128, 512], f32, tag="pb")
        mm(pw[:], warm_sb[:, 0:128], warm_sb[:], start=True, stop=True)

    # ---------------- phases (own + prefix) -> Q, Kpre ----------------
    ph_ps = pa.tile([P, T], f32, tag="pa")
    for kt in range(2):
        mm(ph_ps[:], kw_sb[:, kt, :], xct_sb[:, kt, :],
           start=(kt == 0), stop=(kt == 1))
    # V at odd own tokens (needs only xct+vw; fills the phasor-ACT window)
    for blk in range(2):
        vp = pb.tile([128, D], f32, tag="pb")
        for kt in range(2):
            mm(vp[:], _r(xct_sb[:, kt, 256 * blk + 1: 256 * blk + 256: 2]),
               _r(vw_sb[:, kt, :]), start=(kt == 0), stop=(kt == 1))
        tcp(vodd_sb[:, blk, :], vp[:])
    pp1 = pa.tile([P, 512], f32, tag="pa")
    pp2 = pa.tile([P, 256], f32, tag="pa2", bufs=1)
    for kt in range(2):
        mm(pp1[:], kwb_sb[:, kt, :], xpet_sb[:, kt, 0:512],
           start=(kt == 0), stop=(kt == 1))
    for kt in range(2):
        mm(pp2[:], kwb_sb[:, kt, :], xpet_sb[:, kt, 512:768],
           start=(kt == 0), stop=(kt == 1))
    # own-chunk phasors first (they gate scores); abs runs on DVE.
    # tanh/sin/abs all live in the silu table set: zero reloads.
    act(tv_sb[P:2 * P, :], ph_ps[:], AF.Tanh, bias=qbias_sb[P:2 * P, :])
    act(tv_sb[0:P, :], tv_sb[P:2 * P, :], AF.Abs, bias=qbias_sb[P:2 * P, :])
    act(qb_sb[:], tv_sb[:], AF.Sin, bias=qbias_sb[:], scale=qscale_sb[:])
    act(tvp_sb[P:2 * P, 0:512], pp1[:], AF.Tanh, bias=qbias_sb[P:2 * P, :])
    act(tvp_sb[P:2 * P, 512:768], pp2[:], AF.Tanh, bias=qbias_sb[P:2 * P, :])
    act(tvp_sb[0:P, :], tvp_sb[P:2 * P, :], AF.Abs, bias=qbias_sb[P:2 * P, :])
    act(kpre_sb[:], tvp_sb[:], AF.Sin, bias=qbias_sb[:], scale=qscale_sb[:])
    # preload the Gelu table right after the last Sin: reading kpre pins
    # this dummy after sin-pre so the scheduler can't hoist it (which
    # would evict the silu/sin table set mid-pipeline).
    act(dumm_sb[:], kpre_sb[0:1, 0:1], AF.Gelu, bias=dumm_sb[:])

    # ---------------- scores (odd tk only) + causal mask ----------------
    sc0 = pb.tile([128, 512], f32, tag="pb")
    mm(sc0[:], _r(qb_sb[:, 0:255:2]), _r(qb_sb[:]), start=True, stop=True)
    tt_(ss0_sb[:, 0:256], sc0[:, 0:256], m0_sb[:], OP.mult)
    nc.scalar.copy(ss0_sb[:, 256:512], sc0[:, 256:512])
    sc1 = pa.tile([128, 256], f32, tag="pa")
    mm(sc1[:], _r(qb_sb[:, 256:511:2]), _r(qb_sb[:, 256:512]),
       start=True, stop=True)
    tt_(ss1_sb[:], sc1[:], m0_sb[:], OP.mult)

    # ---------------- Kpre^T, G, S ----------------
    kptp = pa.tile([128, 6, 64], bf16, tag="pa2", bufs=1)
    for j in range(6):
        nc.tensor.transpose(kptp[:, j, :], kpre_sb[:, 128 * j: 128 * j + 128],
                            eyeb_sb[:])
    tcp(kpreT_sb[:], kptp[:])
    for dh in range(2):
        gp = pa.tile([128, 64], f32, tag="pa2", bufs=1)
        for j in range(6):
            mm(gp[:], xpo_sb[:, j, 128 * dh: 128 * dh + 128],
               kpreT_sb[:, j, :], start=(j == 0), stop=(j == 5))
        tcp(gT_sb[:, dh, :], gp[:])
    s_ps = pa.tile([2 * P, D], f32, tag="pa")
    for kt in range(2):
        mm(s_ps[:], _r(gT_sb[:, kt, :]), _r(vw_sb[:, kt, :]),
           start=(kt == 0), stop=(kt == 1))
    nc.scalar.copy(s_sb[:], s_ps[:])

    # ---------------- retrieved^T = V^T ss + S^T Q ----------------
    for dh in range(2):
        cp = pa.tile([128, 512], f32, tag="pa")
        mm(cp[:], _r(s_sb[:, 128 * dh: 128 * dh + 128]), _r(qb_sb[:]),
           start=True, stop=True)
        nc.scalar.copy(cross_sb[:, dh, :], cp[:])
    for dh in range(2):
        rp = pb.tile([128, 512], f32, tag="pb")
        mm(rp[:, 0:256], _r(vodd_sb[:, 0, 128 * dh: 128 * dh + 128]),
           _r(ss0_sb[:, 0:256]), start=True, stop=True)
        mm(rp[:, 256:512], _r(vodd_sb[:, 0, 128 * dh: 128 * dh + 128]),
           _r(ss0_sb[:, 256:512]), start=True, stop=False)
        mm(rp[:, 256:512], _r(vodd_sb[:, 1, 128 * dh: 128 * dh + 128]),
           _r(ss1_sb[:]), start=False, stop=True)
        tt_(r_sb[:, dh, :], rp[:], cross_sb[:, dh, :], OP.add)

    # ---------------- LN stats (squares on GpSimd, no ACT tables) -------
    def ln_stats(src, sq, mean_sb, var_sb, m2v_sb):
        tt_(sq[:], src[:], src[:], OP.mult)
        stp = pa.tile([1, T], f32, tag="pa")
        for kt in range(2):
            mm(stp[:], invdb_sb[:], src[:, kt, :],
               start=(kt == 0), stop=(kt == 1))
        msqp = pa.tile([1, T], f32, tag="pa2", bufs=1)
        for kt in range(2):
            mm(msqp[:], invdb_sb[:], sq[:, kt, :],
               start=(kt == 0), stop=(kt == 1))
        tcp(mean_sb[:], stp[:])
        tt_(m2v_sb[:], mean_sb[:], mean_sb[:], OP.mult)
        stt(out=var_sb[:], in0=msqp[:], scalar=1e-5, in1=m2v_sb[:],
            op0=OP.add, op1=OP.subtract)

    def ln_rstd(var_sb, stdc, rstdc, iters=2):
        # var row -> columns [128,4]; rstd = rsqrt(var) via magic-seed
        # Newton on GpSimd (keeps DVE free), no ACT table loads.
        vc = pt.tile([128, 4], f32, tag="ptr")
        for j in range(4):
            nc.tensor.transpose(vc[:, j: j + 1],
                                var_sb[0:1, 128 * j: 128 * j + 128],
                                eyef_sb[0:1, 0:1])
        i32 = mybir.dt.int32
        tcp(stdc[:], vc[:])
        ts_(out=rstdc[:].bitcast(i32), in0=stdc[:].bitcast(i32), scalar1=1,
            scalar2=None, op0=OP.logical_shift_right)
        ts_(out=rstdc[:].bitcast(i32), in0=rstdc[:].bitcast(i32), scalar1=-1,
            scalar2=0x5F3759DF, op0=OP.mult, op1=OP.add)
        for _ in range(iters):
            tt_(nwt_sb[:], rstdc[:], rstdc[:], OP.mult)
            stt(out=nwt_sb[:], in0=nwt_sb[:], scalar=-0.5, in1=stdc[:],
                op0=OP.mult, op1=OP.mult)
            ts_(out=nwt_sb[:], in0=nwt_sb[:], scalar1=1.5, scalar2=None,
                op0=OP.add)
            tt_(rstdc[:], rstdc[:], nwt_sb[:], OP.mult)

    # W1 on raw r (rstd folded after the matmul):
    #   (W1g^T r - mean*c1n) * rstd == W1g^T(LN1(r))
    def w1_mm(m):
        hp = pb.tile([128, 512], f32, tag="pb")
        for kt in range(2):
            mm(hp[:], w1g_sb[:, kt, 128 * m: 128 * m + 128], r_sb[:, kt, :],
               start=(kt == 0), stop=False)
        return hp

    def w1_fix(hp, m):
        mm(hp[:], c1n_sb[0:1, 128 * m: 128 * m + 128], mean1_sb[:],
           start=False, stop=True)
        return hp

    def w1_block(m):
        return w1_fix(w1_mm(m), m)

    # the first two W1 kt-matmul pairs keep the PE dense while the LN1
    # stats/rows are computed on DVE
    hps = [w1_mm(0)]
    ln_stats(r_sb, sq1_sb, mean1_sb, var1_sb, m2_sb)
    hps.append(w1_mm(1))
    for m in range(2):
        w1_fix(hps[m], m)
    ln_rstd(var1_sb, stdc_sb, rstdc_sb, iters=1)
    # rstd1 broadcast row [128, T]
    rr = pa.tile([1, T], f32, tag="pa")
    for j in range(4):
        nc.tensor.transpose(rr[0:1, 128 * j: 128 * j + 128],
                            rstdc_sb[:, j: j + 1], eyef_sb[:])
    tcp(rstd1_sb[:], rr[:])
    rb1 = pb.tile([128, 512], f32, tag="pb")
    mm(rb1[:], _r(ones_sb[:]), _r(rstd1_sb[:]), start=True, stop=True)
    nc.scalar.copy(rb1s_sb[:], rb1[:])
    for m in range(4):
        if m >= len(hps):
            hps.append(w1_block(m))
        hp = hps[m]
        tt_(hi_sb[:, m, :], hp[:], rb1s_sb[:], OP.mult)
        act(h_sb[:, m, :], hi_sb[:, m, :], AF.Gelu,
            bias=cb1_sb[:, m: m + 1])

    # ---------------- W2 -> refined ----------------
    for dh in range(2):
        fp = pb.tile([128, 512], f32, tag="pb")
        for kt in range(4):
            mm(fp[:], w2_sb[:, kt, 128 * dh: 128 * dh + 128], h_sb[:, kt, :],
               start=(kt == 0), stop=(kt == 3))
        act(f_sb[:, dh, :], fp[:], AF.Identity, bias=b2c_sb[:, dh: dh + 1])

    # ---------------- LN2 + Wo ----------------
    ln_stats(f_sb, sq2_sb, mean2_sb, var2_sb, m2b_sb)

    def wo_block(tm):
        op = pb.tile([128, D], f32, tag="pb")
        for dh in range(2):
            mm(op[:], f_sb[:, dh, 128 * tm: 128 * tm + 128],
               wog_sb[:, dh, :], start=(dh == 0), stop=False)
        mm(op[:], mean2_sb[0:1, 128 * tm: 128 * tm + 128], c1on_sb[:],
           start=False, stop=True)
        return op

    opsb_sb = sb.tile([128, 4, 256], f32)
    ops = [wo_block(0)]
    ln_rstd(var2_sb, stdc2_sb, rstdc2_sb, iters=1)
    for tm in range(4):
        if tm >= len(ops):
            ops.append(wo_block(tm))
        nc.scalar.copy(opsb_sb[:, tm, :], ops[tm][:])
    for tm in range(4):
        stt(out=out_sb[:, tm, :], in0=opsb_sb[:, tm, :],
            scalar=rstdc2_sb[:, tm: tm + 1],
            in1=xc_sb[:, tm, :], op0=OP.mult, op1=OP.add)
        (dma if tm % 2 == 0 else nc.scalar.dma_start)(
            outc[128 * tm: 128 * tm + 128, :], out_sb[:, tm, :])

    pb.release()
    pa.release()
    pt.release()
    sb.release()


_CACHE = {}


def _get_nc():
    if "nc" not in _CACHE:
        _CACHE["nc"] = _build()
    return _CACHE["nc"]


def _bf(a):
    return np.asarray(a, np.float32).astype(ml_dtypes.bfloat16)


def make_in_maps(x, key_W, key_b, val_W, val_b, ln1_g, ln1_b, W1, b1, W2, b2,
                 ln2_g, ln2_b, Wo, bo):
    # these are identically zero for this module; the kernel folds them out
    assert np.allclose(val_b, 0.0), "nonzero val_b unsupported"
    assert np.allclose(key_b, 0.0), "nonzero key_b unsupported"
    assert np.allclose(bo + ln2_b @ Wo, 0.0), "nonzero output bias unsupported"

    def pm(a, k):
        # [k*128, F] row-major -> partition-major [128, k, F]
        a = np.ascontiguousarray(a)
        return np.ascontiguousarray(
            a.reshape(k, 128, -1).transpose(1, 0, 2))

    w1g = ln1_g[:, None] * W1
    wog = ln2_g[:, None] * Wo
    shared = {
        "kw": pm(np.asarray(key_W, np.float32), 2),
        "kwb": pm(_bf(key_W), 2),
        "vw": pm(np.asarray(val_W, np.float32), 2),
        "w1g": pm(_bf(w1g), 2), "c1n": _bf(-w1g.sum(0, keepdims=True)),
        "cb1": np.ascontiguousarray(
            (b1 + ln1_b @ W1).reshape(4, 128).T),
        "w2": pm(_bf(W2), 4), "b2c": np.ascontiguousarray(
            b2.reshape(2, 128).T),
        "wog": pm(_bf(wog), 2), "c1on": _bf(-wog.sum(0, keepdims=True)),
        "m0": _bf((np.arange(1, 256, 2)[:, None] <=
                   np.arange(256)[None, :]).astype(np.float32)),
        "eyef": np.eye(128, dtype=np.float32),
        "eyeb": _bf(np.eye(64)),
        "onesr": np.ones((1, 128), np.float32),
        "invdb": _bf(np.full((128, 1), 1.0 / D, np.float32)),
    }
    in_maps = []
    for i in range(N_CORES):
        b, c = divmod(i, 4)
        l0 = c * T
        npairs = l0 // 2
        xpet = np.zeros((D, PRE), np.float32)
        xpo = np.zeros((PRE, D), np.float32)
        if npairs:
            xpet[:, :npairs] = x[b, 0:l0 - 1:2].T
            xpo[:npairs] = x[b, 1:l0:2]
        in_maps.append({
            "xc": pm(x[b, l0:l0 + T], 4),
            "xct": pm(np.ascontiguousarray(x[b, l0:l0 + T].T), 2),
            "xpet": pm(_bf(xpet), 2), "xpo": pm(_bf(xpo), 6), **shared,
        })
    return in_maps


def kernel(**inputs):
    inputs = {k: np.asarray(v, np.float32) for k, v in inputs.items()}
    in_maps = make_in_maps(**inputs)
    nc = _get_nc()
    res = run_bass_kernel_spmd(nc, in_maps, core_ids=list(range(N_CORES)),
                               **_CACHE.get("run_kwargs", {}))
    _CACHE["last_result"] = res
    out = np.empty((B, L, D), np.float32)
    for i in range(N_CORES):
        b, c = divmod(i, 4)
        out[b, c * T:(c + 1) * T] = res.results[i]["outc"]
    return out


# revision 35
# speedup vs baseline: 1.1703x; 1.1703x over previous
"""Trainium2 Bass kernel for nn_HardcodedKVMemoryBlock (8 NeuronCores).

Sharding: core i handles batch b=i//4, sequence chunk c=i%4 (512 tokens).
The (B,L,P,D) cumsum is restructured as causal linear attention:
    retrieved = tril(Q K^T) @ V + Q @ S_prefix
with Q=[cos,sin] phasors (L x 64), V = values at odd positions, and the
cross-chunk carry S_prefix computed redundantly per core from a
zero-padded prefix (no collectives). The 1/sqrt(valid*P) normalization
cancels inside LayerNorm1 (scale invariance); ln gains are folded into
W1/Wo and means are applied as rank-1 PE updates.

v3 (structural rework of the v1 baseline):
 - x and the prefix-even tokens are pre-transposed on the host, removing
   all 20 PE layout transposes + psum copies of v1.
 - squares for LN stats run on DVE/GpSimd (Square table loads gone).
 - activation tables (tanh/gelu) are prefetched with dummy ACTs so no
   table load sits on the critical path.
 - cos = sin(-pi*|t| + pi/2) via per-partition scale/bias APs, removing
   the affine DVE pass of v1.
 - rsqrt Newton iterations run on GpSimd (DVE freed).
 - dummy warmup matmuls keep the PE busy during the input DMA so the
   tensor engine p-state is ramped when real work arrives.
"""

import math
import numpy as np
import ml_dtypes

import concourse.bass as bass
import concourse.tile as tile
from concourse import bacc, mybir
from concourse.bass_utils import run_bass_kernel_spmd

PI = math.pi
B, L, D, P = 2, 2048, 256, 32
T = 512          # own tokens per core
H = 512          # MLP hidden
PRE = 768        # padded prefix pair count (max prefix 1536 tokens / 2)
N_CORES = 8

f32 = mybir.dt.float32
f32r = mybir.dt.float32r
bf16 = mybir.dt.bfloat16
AF = mybir.ActivationFunctionType
OP = mybir.AluOpType


def _r(ap):
    return ap.bitcast(f32r)


N_WARM = 3       # dummy PE warmup matmuls


def _build():
    nc = bacc.Bacc("TRN2", target_bir_lowering=False, debug=False,
                   num_devices=N_CORES)

    def din(name, shape, dt):
        return nc.dram_tensor(name, shape, dt, kind="ExternalInput").ap()

    # all inputs are stored partition-major on the host so every DMA is
    # 128 contiguous rows (minimal descriptors -> cheap issue, fast xfer)
    xc = din("xc", [128, 4, 256], f32)     # own tokens token-major (residual)
    xct = din("xct", [128, 2, 512], f32r)  # own tokens, transposed
    xpet = din("xpet", [128, 2, PRE], bf16)
    xpo = din("xpo", [128, 6, 256], bf16)
    kw = din("kw", [128, 2, P], f32r)
    kwb = din("kwb", [128, 2, P], bf16)
    vw = din("vw", [128, 2, 256], f32r)
    w1g = din("w1g", [128, 2, H], bf16)    # ln1_g . W1
    c1n = din("c1n", [1, H], bf16)         # -w1g.sum(0)
    cb1 = din("cb1", [128, 4], f32)        # b1 + ln1_b@W1, col layout
    w2 = din("w2", [128, 4, 256], bf16)
    b2c = din("b2c", [128, 2], f32)
    wog = din("wog", [128, 2, 256], bf16)  # ln2_g . Wo
    c1on = din("c1on", [1, D], bf16)       # -wog.sum(0)
    m0 = din("m0", [128, 256], bf16)       # causal mask for odd writes
    onesr = din("onesr", [1, 128], f32r)
    invdb = din("invdb", [128, 1], bf16)   # 1/D
    eyef = din("eyef", [128, 128], f32)
    eyeb = din("eyeb", [64, 64], bf16)
    outc = nc.dram_tensor("outc", [T, D], f32, kind="ExternalOutput").ap()

    with tile.TileContext(nc) as tc:
        _emit(tc, locals())
    nc.compile()
    return nc


def _emit(tc, io):
    nc = tc.nc
    outc = io["outc"]

    sb = tc.alloc_tile_pool(name="sb", bufs=1)
    pt = tc.alloc_tile_pool(name="pt", bufs=2, space="PSUM")    # small tiles
    pa = tc.alloc_tile_pool(name="pa", bufs=2, space="PSUM")    # narrow tiles
    pb = tc.alloc_tile_pool(name="pb", bufs=3, space="PSUM")    # [128,512]

    # ---------------- constant / weight tiles ----------------
    xc_sb = sb.tile([128, 4, 256], f32)
    xct_sb = sb.tile([128, 2, 512], f32r)
    xpet_sb = sb.tile([128, 2, PRE], bf16)
    xpo_sb = sb.tile([128, 6, 256], bf16)
    kw_sb = sb.tile([128, 2, P], f32r)
    kwb_sb = sb.tile([128, 2, P], bf16)
    vw_sb = sb.tile([128, 2, 256], f32r)
    w1g_sb = sb.tile([128, 2, H], bf16)
    c1n_sb = sb.tile([1, H], bf16)
    cb1_sb = sb.tile([128, 4], f32)
    w2_sb = sb.tile([128, 4, 256], bf16)
    b2c_sb = sb.tile([128, 2], f32)
    wog_sb = sb.tile([128, 2, 256], bf16)
    c1on_sb = sb.tile([1, D], bf16)
    m0_sb = sb.tile([128, 256], bf16)
    eyef_sb = sb.tile([128, 128], f32)
    eyeb_sb = sb.tile([64, 64], bf16)
    ones_sb = sb.tile([1, 128], f32r)
    invdb_sb = sb.tile([128, 1], bf16)
    qbias_sb = sb.tile([2 * P, 1], f32)    # pi/2 rows 0:32, 0 rows 32:64
    qscale_sb = sb.tile([2 * P, 1], f32)   # -pi rows 0:32, pi rows 32:64
    zero128_sb = sb.tile([128, 1], f32)
    halfpi_sb = sb.tile([128, 1], f32)
    dumm_sb = sb.tile([1, 1], f32)
    warm_sb = sb.tile([128, 512], bf16)

    dma = nc.sync.dma_start
    dmaw = nc.gpsimd.dma_start
    nc.vector.memset(qbias_sb[0:P, :], PI / 2)
    nc.vector.memset(qbias_sb[P:2 * P, :], 0.0)
    nc.vector.memset(qscale_sb[0:P, :], -PI)
    nc.vector.memset(qscale_sb[P:2 * P, :], PI)
    nc.vector.memset(zero128_sb[:], 0.0)
    nc.vector.memset(halfpi_sb[:], PI / 2)
    nc.vector.memset(dumm_sb[:], 0.0)
    nc.vector.memset(warm_sb[:], 0.0)

    mm = nc.tensor.matmul
    act = nc.scalar.activation
    tt_ = nc.vector.tensor_tensor
    tcp = nc.vector.tensor_copy
    ts_ = nc.vector.tensor_scalar
    stt = nc.vector.scalar_tensor_tensor
    gts = nc.gpsimd.tensor_scalar
    gstt = nc.gpsimd.scalar_tensor_tensor
    gtt = nc.gpsimd.tensor_tensor

    # preload the silu_and_others table set (contains silu+tanh+sin+abs+copy)
    # while DMAs run: a Silu dummy forces that exact set, so the whole
    # phasor pipeline below needs zero further table loads.
    act(dumm_sb[:], dumm_sb[:], AF.Silu, bias=dumm_sb[:])

    # ---------------- data tiles ----------------
    tv_sb = sb.tile([2 * P, T], f32)       # rows 0:32 |tanh|, 32:64 tanh
    qb_sb = sb.tile([2 * P, T], f32r)       # rows 0:32 cos, 32:64 sin
    tvp_sb = sb.tile([2 * P, PRE], f32)
    kpre_sb = sb.tile([2 * P, PRE], bf16)
    kpreT_sb = sb.tile([128, 6, 64], bf16)
    gT_sb = sb.tile([128, 2, 64], f32r)
    s_sb = sb.tile([2 * P, D], f32r)        # prefix state S
    vodd_sb = sb.tile([128, 2, 256], f32r)  # values at odd tokens
    ss0_sb = sb.tile([128, 512], f32r)
    ss1_sb = sb.tile([128, 256], f32r)
    cross_sb = sb.tile([128, 2, 512], f32)
    r_sb = sb.tile([128, 2, 512], bf16)    # retrieved
    sq1_sb = sb.tile([128, 2, 512], bf16)
    mean1_sb = sb.tile([1, T], bf16)
    var1_sb = sb.tile([1, T], f32)
    m2_sb = sb.tile([1, T], f32)
    rstd1_sb = sb.tile([1, T], f32r)
    rb1s_sb = sb.tile([128, T], f32)
    hi_sb = sb.tile([128, 4, 512], bf16)
    h_sb = sb.tile([128, 4, 512], bf16)
    f_sb = sb.tile([128, 2, 512], bf16)    # refined
    sq2_sb = sb.tile([128, 2, 512], bf16)
    mean2_sb = sb.tile([1, T], bf16)
    var2_sb = sb.tile([1, T], f32)
    m2b_sb = sb.tile([1, T], f32)
    stdc_sb = sb.tile([128, 4], f32)
    rstdc_sb = sb.tile([128, 4], f32)
    stdc2_sb = sb.tile([128, 4], f32)
    rstdc2_sb = sb.tile([128, 4], f32)
    nwt_sb = sb.tile([128, 4], f32)
    out_sb = sb.tile([128, 4, 256], f32)

    from concourse.tile import add_dep_helper  # noqa: E402
    # critical-path loads first; prefix tensors issue from the gpsimd queue
    # xct split across 2 DMA rings - a single ring moves only ~90GB/s
    dma(xct_sb[:, 0, 0:256], io["xct"][:, 0, 0:256])
    dma(xct_sb[:, 0, 256:512], io["xct"][:, 0, 256:512])
    dma(xct_sb[:, 1, 0:256], io["xct"][:, 1, 0:256])
    dma(xct_sb[:, 1, 256:512], io["xct"][:, 1, 256:512])
    dma(kw_sb[:], io["kw"])
    dma(kwb_sb[:], io["kwb"])
    d_xpet = dmaw(xpet_sb[:], io["xpet"])
    dma(vw_sb[:], io["vw"])
    d_xpo = dmaw(xpo_sb[:], io["xpo"])
    dma(m0_sb[:], io["m0"])
    dma(invdb_sb[:], io["invdb"])
    dma(eyeb_sb[:], io["eyeb"])
    wd = []
    wd.append(dmaw(w1g_sb[:], io["w1g"]))
    wd.append(dmaw(c1n_sb[:], io["c1n"]))
    wd.append(dmaw(cb1_sb[:], io["cb1"]))
    wd.append(dmaw(w2_sb[:], io["w2"]))
    wd.append(dmaw(b2c_sb[:], io["b2c"]))
    wd.append(dmaw(wog_sb[:], io["wog"]))
    wd.append(dmaw(c1on_sb[:], io["c1on"]))
    wd.append(dmaw(eyef_sb[:], io["eyef"]))
    wd.append(dmaw(ones_sb[:], io["onesr"]))
    d_xc = dmaw(xc_sb[:], io["xc"])
    # bulk weights wait for the latency-critical input loads to finish so
    # they don't steal HBM bandwidth from the critical path
    for w in wd:
        add_dep_helper(w.ins, d_xpo.ins, sync=True,
                       reason="bulk weights after data loads")
    add_dep_helper(d_xc.ins, d_xpo.ins, sync=True,
                   reason="residual x after data loads")

    # ---------------- PE warmup (ramps the p-state during DMA) ----------
    for w in range(N_WARM):
        pw = pb.tile([128, 512], f32, tag="pb")
        mm(pw[:], warm_sb[:, 0:128], warm_sb[:], start=True, stop=True)

    # ---------------- phases (own + prefix) -> Q, Kpre ----------------
    ph_ps = pa.tile([P, T], f32, tag="pa")
    for kt in range(2):
        mm(ph_ps[:], kw_sb[:, kt, :], xct_sb[:, kt, :],
           start=(kt == 0), stop=(kt == 1))
    # V at odd own tokens (needs only xct+vw; fills the phasor-ACT window)
    for blk in range(2):
        vp = pb.tile([128, D], f32, tag="pb")
        for kt in range(2):
            mm(vp[:], _r(xct_sb[:, kt, 256 * blk + 1: 256 * blk + 256: 2]),
               _r(vw_sb[:, kt, :]), start=(kt == 0), stop=(kt == 1))
        tcp(vodd_sb[:, blk, :], vp[:])
    pp1 = pa.tile([P, 512], f32, tag="pa")
    pp2 = pa.tile([P, 256], f32, tag="pa2", bufs=1)
    for kt in range(2):
        mm(pp1[:], kwb_sb[:, kt, :], xpet_sb[:, kt, 0:512],
           start=(kt == 0), stop=(kt == 1))
    for kt in range(2):
        mm(pp2[:], kwb_sb[:, kt, :], xpet_sb[:, kt, 512:768],
           start=(kt == 0), stop=(kt == 1))
    # own-chunk phasors first (they gate scores); abs runs on DVE.
    # tanh/sin/abs all live in the silu table set: zero reloads.
    act(tv_sb[P:2 * P, :], ph_ps[:], AF.Tanh, bias=qbias_sb[P:2 * P, :])
    act(tv_sb[0:P, :], tv_sb[P:2 * P, :], AF.Abs, bias=qbias_sb[P:2 * P, :])
    act(qb_sb[:], tv_sb[:], AF.Sin, bias=qbias_sb[:], scale=qscale_sb[:])
    act(tvp_sb[P:2 * P, 0:512], pp1[:], AF.Tanh, bias=qbias_sb[P:2 * P, :])
    act(tvp_sb[P:2 * P, 512:768], pp2[:], AF.Tanh, bias=qbias_sb[P:2 * P, :])
    act(tvp_sb[0:P, :], tvp_sb[P:2 * P, :], AF.Abs, bias=qbias_sb[P:2 * P, :])
    act(kpre_sb[:], tvp_sb[:], AF.Sin, bias=qbias_sb[:], scale=qscale_sb[:])
    # preload the Gelu table right after the last Sin: reading kpre pins
    # this dummy after sin-pre so the scheduler can't hoist it (which
    # would evict the silu/sin table set mid-pipeline).
    act(dumm_sb[:], kpre_sb[0:1, 0:1], AF.Gelu, bias=dumm_sb[:])

    # ---------------- scores (odd tk only) + causal mask ----------------
    sc0 = pb.tile([128, 512], f32, tag="pb")
    mm(sc0[:], _r(qb_sb[:, 0:255:2]), _r(qb_sb[:]), start=True, stop=True)
    tt_(ss0_sb[:, 0:256], sc0[:, 0:256], m0_sb[:], OP.mult)
    nc.scalar.copy(ss0_sb[:, 256:512], sc0[:, 256:512])
    sc1 = pa.tile([128, 256], f32, tag="pa")
    mm(sc1[:], _r(qb_sb[:, 256:511:2]), _r(qb_sb[:, 256:512]),
       start=True, stop=True)
    tt_(ss1_sb[:], sc1[:], m0_sb[:], OP.mult)

    # ---------------- Kpre^T, G, S ----------------
    kptp = pa.tile([128, 6, 64], bf16, tag="pa2", bufs=1)
    for j in range(6):
        nc.tensor.transpose(kptp[:, j, :], kpre_sb[:, 128 * j: 128 * j + 128],
                            eyeb_sb[:])
    tcp(kpreT_sb[:], kptp[:])
    for dh in range(2):
        gp = pa.tile([128, 64], f32, tag="pa2", bufs=1)
        for j in range(6):
            mm(gp[:], xpo_sb[:, j, 128 * dh: 128 * dh + 128],
               kpreT_sb[:, j, :], start=(j == 0), stop=(j == 5))
        tcp(gT_sb[:, dh, :], gp[:])
    s_ps = pa.tile([2 * P, D], f32, tag="pa")
    for kt in range(2):
        mm(s_ps[:], _r(gT_sb[:, kt, :]), _r(vw_sb[:, kt, :]),
           start=(kt == 0), stop=(kt == 1))
    nc.scalar.copy(s_sb[:], s_ps[:])

    # ---------------- retrieved^T = V^T ss + S^T Q ----------------
    for dh in range(2):
        cp = pa.tile([128, 512], f32, tag="pa")
        mm(cp[:], _r(s_sb[:, 128 * dh: 128 * dh + 128]), _r(qb_sb[:]),
           start=True, stop=True)
        nc.scalar.copy(cross_sb[:, dh, :], cp[:])
    for dh in range(2):
        rp = pb.tile([128, 512], f32, tag="pb")
        mm(rp[:, 0:256], _r(vodd_sb[:, 0, 128 * dh: 128 * dh + 128]),
           _r(ss0_sb[:, 0:256]), start=True, stop=True)
        mm(rp[:, 256:512], _r(vodd_sb[:, 0, 128 * dh: 128 * dh + 128]),
           _r(ss0_sb[:, 256:512]), start=True, stop=False)
        mm(rp[:, 256:512], _r(vodd_sb[:, 1, 128 * dh: 128 * dh + 128]),
           _r(ss1_sb[:]), start=False, stop=True)
        tt_(r_sb[:, dh, :], rp[:], cross_sb[:, dh, :], OP.add)

    # ---------------- LN stats (squares on GpSimd, no ACT tables) -------
    def ln_stats(src, sq, mean_sb, var_sb, m2v_sb):
        tt_(sq[:], src[:], src[:], OP.mult)
        stp = pa.tile([1, T], f32, tag="pa")
        for kt in range(2):
            mm(stp[:], invdb_sb[:], src[:, kt, :],
               start=(kt == 0), stop=(kt == 1))
        msqp = pa.tile([1, T], f32, tag="pa2", bufs=1)
        for kt in range(2):
            mm(msqp[:], invdb_sb[:], sq[:, kt, :],
               start=(kt == 0), stop=(kt == 1))
        tcp(mean_sb[:], stp[:])
        tt_(m2v_sb[:], mean_sb[:], mean_sb[:], OP.mult)
        stt(out=var_sb[:], in0=msqp[:], scalar=1e-5, in1=m2v_sb[:],
            op0=OP.add, op1=OP.subtract)

    def ln_rstd(var_sb, stdc, rstdc, iters=2):
        # var row -> columns [128,4]; rstd = rsqrt(var) via magic-seed
        # Newton on GpSimd (keeps DVE free), no ACT table loads.
        vc = pt.tile([128, 4], f32, tag="ptr")
        for j in range(4):
            nc.tensor.transpose(vc[:, j: j + 1],
                                var_sb[0:1, 128 * j: 128 * j + 128],
                                eyef_sb[0:1, 0:1])
        i32 = mybir.dt.int32
        tcp(stdc[:], vc[:])
        ts_(out=rstdc[:].bitcast(i32), in0=stdc[:].bitcast(i32), scalar1=1,
            scalar2=None, op0=OP.logical_shift_right)
        ts_(out=rstdc[:].bitcast(i32), in0=rstdc[:].bitcast(i32), scalar1=-1,
            scalar2=0x5F3759DF, op0=OP.mult, op1=OP.add)
        for _ in range(iters):
            tt_(nwt_sb[:], rstdc[:], rstdc[:], OP.mult)
            stt(out=nwt_sb[:], in0=nwt_sb[:], scalar=-0.5, in1=stdc[:],
                op0=OP.mult, op1=OP.mult)
            ts_(out=nwt_sb[:], in0=nwt_sb[:], scalar1=1.5, scalar2=None,
                op0=OP.add)
            tt_(rstdc[:], rstdc[:], nwt_sb[:], OP.mult)

    # W1 on raw r (rstd folded after the matmul):
    #   (W1g^T r - mean*c1n) * rstd == W1g^T(LN1(r))
    def w1_mm(m):
        hp = pb.tile([128, 512], f32, tag="pb")
        for kt in range(2):
            mm(hp[:], w1g_sb[:, kt, 128 * m: 128 * m + 128], r_sb[:, kt, :],
               start=(kt == 0), stop=False)
        return hp

    def w1_fix(hp, m):
        mm(hp[:], c1n_sb[0:1, 128 * m: 128 * m + 128], mean1_sb[:],
           start=False, stop=True)
        return hp

    def w1_block(m):
        return w1_fix(w1_mm(m), m)

    # the first two W1 kt-matmul pairs keep the PE dense while the LN1
    # stats/rows are computed on DVE
    hps = [w1_mm(0)]
    ln_stats(r_sb, sq1_sb, mean1_sb, var1_sb, m2_sb)
    hps.append(w1_mm(1))
    for m in range(2):
        w1_fix(hps[m], m)
    ln_rstd(var1_sb, stdc_sb, rstdc_sb, iters=1)
    # rstd1 broadcast row [128, T]
    rr = pa.tile([1, T], f32, tag="pa")
    for j in range(4):
        nc.tensor.transpose(rr[0:1, 128 * j: 128 * j + 128],
                            rstdc_sb[:, j: j + 1], eyef_sb[:])
    tcp(rstd1_sb[:], rr[:])
    rb1 = pb.tile([128, 512], f32, tag="pb")
    mm(rb1[:], _r(ones_sb[:]), _r(rstd1_sb[:]), start=True, stop=True)
    nc.scalar.copy(rb1s_sb[:], rb1[:])
    for m in range(4):
        if m >= len(hps):
            hps.append(w1_block(m))
        hp = hps[m]
        tt_(hi_sb[:, m, :], hp[:], rb1s_sb[:], OP.mult)
        act(h_sb[:, m, :], hi_sb[:, m, :], AF.Gelu,
            bias=cb1_sb[:, m: m + 1])

    # ---------------- W2 -> refined ----------------
    for dh in range(2):
        fp = pb.tile([128, 512], f32, tag="pb")
        for kt in range(4):
            mm(fp[:], w2_sb[:, kt, 128 * dh: 128 * dh + 128], h_sb[:, kt, :],
               start=(kt == 0), stop=(kt == 3))
        act(f_sb[:, dh, :], fp[:], AF.Identity, bias=b2c_sb[:, dh: dh + 1])

    # ---------------- LN2 + Wo ----------------
    ln_stats(f_sb, sq2_sb, mean2_sb, var2_sb, m2b_sb)

    def wo_block(tm):
        op = pb.tile([128, D], f32, tag="pb")
        for dh in range(2):
            mm(op[:], f_sb[:, dh, 128 * tm: 128 * tm + 128],
               wog_sb[:, dh, :], start=(dh == 0), stop=False)
        mm(op[:], mean2_sb[0:1, 128 * tm: 128 * tm + 128], c1on_sb[:],
           start=False, stop=True)
        return op

    opsb_sb = sb.tile([128, 4, 256], f32)
    ops = [wo_block(0)]
    ln_rstd(var2_sb, stdc2_sb, rstdc2_sb, iters=1)
    for tm in range(4):
        if tm >= len(ops):
            ops.append(wo_block(tm))
        nc.scalar.copy(opsb_sb[:, tm, :], ops[tm][:])
    for tm in range(4):
        stt(out=out_sb[:, tm, :], in0=opsb_sb[:, tm, :],
            scalar=rstdc2_sb[:, tm: tm + 1],
            in1=xc_sb[:, tm, :], op0=OP.mult, op1=OP.add)
        (dma if tm % 2 == 0 else nc.scalar.dma_start)(
            outc[128 * tm: 128 * tm + 128, :], out_sb[:, tm, :])

    pb.release()
    pa.release()
    pt.release()
    sb.release()


_CACHE = {}


def _get_nc():
    if "nc" not in _CACHE:
        _CACHE["nc"] = _build()
    return _CACHE["nc"]


def _bf(a):
    return np.asarray(a, np.float32).astype(ml_dtypes.bfloat16)


def make_in_maps(x, key_W, key_b, val_W, val_b, ln1_g, ln1_b, W1, b1, W2, b2,
                 ln2_g, ln2_b, Wo, bo):
    # these are identically zero for this module; the kernel folds them out
    assert np.allclose(val_b, 0.0), "nonzero val_b unsupported"
    assert np.allclose(key_b, 0.0), "nonzero key_b unsupported"
    assert np.allclose(bo + ln2_b @ Wo, 0.0), "nonzero output bias unsupported"

    def pm(a, k):
        # [k*128, F] row-major -> partition-major [128, k, F]
        a = np.ascontiguousarray(a)
        return np.ascontiguousarray(
            a.reshape(k, 128, -1).transpose(1, 0, 2))

    w1g = ln1_g[:, None] * W1
    wog = ln2_g[:, None] * Wo
    shared = {
        "kw": pm(np.asarray(key_W, np.float32), 2),
        "kwb": pm(_bf(key_W), 2),
        "vw": pm(np.asarray(val_W, np.float32), 2),
        "w1g": pm(_bf(w1g), 2), "c1n": _bf(-w1g.sum(0, keepdims=True)),
        "cb1": np.ascontiguousarray(
            (b1 + ln1_b @ W1).reshape(4, 128).T),
        "w2": pm(_bf(W2), 4), "b2c": np.ascontiguousarray(
            b2.reshape(2, 128).T),
        "wog": pm(_bf(wog), 2), "c1on": _bf(-wog.sum(0, keepdims=True)),
        "m0": _bf((np.arange(1, 256, 2)[:, None] <=
                   np.arange(256)[None, :]).astype(np.float32)),
        "eyef": np.eye(128, dtype=np.float32),
        "eyeb": _bf(np.eye(64)),
        "onesr": np.ones((1, 128), np.float32),
        "invdb": _bf(np.full((128, 1), 1.0 / D, np.float32)),
    }
    in_maps = []
    for i in range(N_CORES):
        b, c = divmod(i, 4)
        l0 = c * T
        npairs = l0 // 2
        xpet = np.zeros((D, PRE), np.float32)
        xpo = np.zeros((PRE, D), np.float32)
        if npairs:
            xpet[:, :npairs] = x[b, 0:l0 - 1:2].T
            xpo[:npairs] = x[b, 1:l0:2]
        in_maps.append({
            "xc": pm(x[b, l0:l0 + T], 4),
            "xct": pm(np.ascontiguousarray(x[b, l0:l0 + T].T), 2),
            "xpet": pm(_bf(xpet), 2), "xpo": pm(_bf(xpo), 6), **shared,
        })
    return in_maps


def kernel(**inputs):
    inputs = {k: np.asarray(v, np.float32) for k, v in inputs.items()}
    in_maps = make_in_maps(**inputs)
    nc = _get_nc()
    res = run_bass_kernel_spmd(nc, in_maps, core_ids=list(range(N_CORES)),
                               **_CACHE.get("run_kwargs", {}))
    _CACHE["last_result"] = res
    out = np.empty((B, L, D), np.float32)
    for i in range(N_CORES):
        b, c = divmod(i, 4)
        out[b, c * T:(c + 1) * T] = res.results[i]["outc"]
    return out
